# revision 23
# baseline (speedup 1.0000x reference)
"""Multi-head attention kernel for Trainium2 (8 NeuronCores).

Problem: B=4, T=2048, U=1024, H=16 heads, D=64. Full (non-causal) softmax
attention per head. 64 independent (head, batch) problems.

Sharding: core c owns batch b = c//2 and head block hb = c%2 (8 contiguous
heads = 512 contiguous channels). No cross-core communication.

Per-core algorithm (T=2048, DD=512 channels, 8 local heads of D=64):
  - Q, K: loaded fp32, converted to bf16 on GpSimd, then transposed into
    QT/KT [d, t] tiles (two heads per 128-partition tile). Head-pair 0 goes
    through fast PE transposes (so compute starts early); pairs 1-3 bounce
    through a bf16 DRAM scratch and use big DMA xbar transposes.
  - V is converted to bf16 into a per-t-chunk [128, 8*65] layout where each
    head's 64 columns are augmented with a ones column (computes the
    softmax denominator for free in the second matmul).
  - Main loop processes a HEAD PAIR at a time so the two mm1 matmuls
    (K=64 contraction each) run CONCURRENTLY in the PE array as row-tiled
    matmuls at tile_position (0,0) and (64,0), writing the two halves
    (= two PSUM banks) of one sc[128, 1024] tile:
      for hp (4 head pairs), qq (4 q-quarters of 512):
        for kc (16 k-chunks of 128):
          mm1 pair: sc[:, 0:512]    = KT_A.T @ QT_A   (concurrent,
                    sc[:, 512:1024] = KT_B.T @ QT_B    row-tiled)
          exp:      pb = exp(sc / 8)  (one ACT instr, FD=1024, bf16)
          mm2 (sw-pipelined `depth` chunks behind mm1/exp so the PE
          never queues behind the current chunk's exp):
                    outpA[65, 512] += V_A_aug[kc].T @ pb[:, 0:512]
                    outpB[65, 512] += V_B_aug[kc].T @ pb[:, 512:1024]
        norm/store per head: r = 1/outp[64]; partition-broadcast (GpSimd);
        out = outp[0:64] * r; split fp32 -> bf16 hi+lo; DMA-transpose both
        to [q, d]; DVE re-add to fp32; store.
  The ACT engine (exp at 1 elem/lane/cycle) is the bottleneck engine;
  PE work per chunk (~0.64us) fits under the exp (~1.03us).
"""

import os
import sys

sys.path.insert(0, "/opt/trn_rl_repo")

import ml_dtypes
import numpy as np

import concourse.bass as bass
import concourse.bacc as bacc
import concourse.mybir as mybir
import concourse.tile as tile
from concourse import library_config
from concourse.bass_utils import run_bass_kernel_spmd

F32 = mybir.dt.float32
BF16 = mybir.dt.bfloat16
EXP = mybir.ActivationFunctionType.Exp

B, T, U = 4, 2048, 1024
H_TOTAL, D = 16, 64
DD = 512          # channels per core (8 heads)
H = 8             # heads per core
KC = 16           # k chunks of 128
TC = 16           # t chunks of 128
HP = 4            # head pairs
NQQ = 4           # q quarters
QQ = 512          # q per quarter
N_CORES = 8
SCALE = 1.0 / 8.0  # 1/sqrt(D)

DEPTH_DEFAULT = int(os.environ.get("K_DEPTH", "3"))


def build_program(nc, bench_iters=0, stages=("mm1", "exp", "mm2", "norm"),
                  depth=None):
    if depth is None:
        depth = DEPTH_DEFAULT
    if bench_iters:
        # Timing-only variant: big tensors are Internal (values irrelevant),
        # external I/O is tiny, and the whole body runs in a For_i loop.
        in_flag = nc.dram_tensor("in_flag", [1, 1], F32, kind="ExternalInput").ap()
        out_flag = nc.dram_tensor("out_flag", [1, 1], F32, kind="ExternalOutput").ap()
        q_d = nc.dram_tensor("querys", [T, DD], F32).ap()
        k_d = nc.dram_tensor("keys", [T, DD], F32).ap()
        v_d = nc.dram_tensor("values", [T, DD], F32).ap()
        id_d = nc.dram_tensor("ident", [128, 128], BF16).ap()
        o_d = nc.dram_tensor("out", [T, DD], F32).ap()
    else:
        q_d = nc.dram_tensor("querys", [T, DD], F32, kind="ExternalInput").ap()
        k_d = nc.dram_tensor("keys", [T, DD], F32, kind="ExternalInput").ap()
        v_d = nc.dram_tensor("values", [T, DD], F32, kind="ExternalInput").ap()
        id_d = nc.dram_tensor("ident", [128, 128], BF16, kind="ExternalInput").ap()
        o_d = nc.dram_tensor("out", [T, DD], F32, kind="ExternalOutput").ap()
    qbf_d = nc.dram_tensor("qbf_scratch", [T, DD], BF16).ap()
    kbf_d = nc.dram_tensor("kbf_scratch", [T, DD], BF16).ap()

    import contextlib

    with tile.TileContext(nc) as tc:
        if bench_iters:
            nc.sync.dma_start(o_d[0:1, 0:1], in_flag[:])  # consume input
            loop_cm = tc.For_i(0, bench_iters, 1)
        else:
            loop_cm = contextlib.nullcontext()
        with (
            tc.tile_pool(name="persist", bufs=1) as persist,
            tc.tile_pool(name="stage", bufs=4) as stage,
            tc.tile_pool(name="probs", bufs=max(3, depth + 2)) as probs_pool,
            tc.tile_pool(name="norm", bufs=2) as norm_pool,
            tc.tile_pool(name="ps_sce", bufs=1, space=bass.MemorySpace.PSUM) as ps_sce,
            tc.tile_pool(name="ps", bufs=1, space=bass.MemorySpace.PSUM) as ps,
            tc.tile_pool(name="ps_sco", bufs=1, space=bass.MemorySpace.PSUM) as ps_sco,
            tc.tile_pool(name="pst", bufs=2, space=bass.MemorySpace.PSUM) as pst,
        ):
            with loop_cm:
                nc.gpsimd.load_library(library_config.attn)

                # Dummy exp to hoist the ACT table load to t=0.
                warm = persist.tile([1, 1], F32, tag="warm")
                nc.gpsimd.memset(warm[:], 0.0)
                warm_o = persist.tile([1, 1], F32, tag="warm_o")
                nc.scalar.activation(warm_o[:], warm[:], EXP)

                ident = persist.tile([128, 128], BF16, tag="ident")
                nc.sync.dma_start(ident[:], id_d[:])

                # persistent tiles
                vc = [
                    persist.tile([128, H * 65], BF16, tag=f"vc{c}", name=f"vc{c}")
                    for c in range(TC)
                ]
                for c in range(TC):
                    nc.gpsimd.memset(
                        vc[c][:].rearrange("p (h e) -> p h e", e=65)[:, :, 64:65], 1.0
                    )
                qt = [
                    persist.tile([128, T], BF16, tag=f"qt{hp}", name=f"qt{hp}")
                    for hp in range(HP)
                ]
                kt = [
                    persist.tile([128, T], BF16, tag=f"kt{hp}", name=f"kt{hp}")
                    for hp in range(HP)
                ]
                v_3d = v_d.rearrange("(c p) d -> c p d", p=128)
                q3s = q_d.rearrange("(c p) d -> c p d", p=128)
                q3d = qbf_d.rearrange("(c p) d -> c p d", p=128)
                k3s = k_d.rearrange("(c p) d -> c p d", p=128)
                k3d = kbf_d.rearrange("(c p) d -> c p d", p=128)

                kbf_tiles = {}
                qbf_tiles = {}

                def qk_chunk(src3, dst3, c, which):
                    s = stage.tile([128, DD], F32, tag="qkstage", name=f"{which}s{c}")
                    nc.sync.dma_start(s[:], src3[c])
                    sb = stage.tile([128, DD], BF16, tag="qkbf", bufs=6,
                                    name=f"{which}b{c}")
                    nc.gpsimd.tensor_copy(sb[:], s[:])
                    nc.sync.dma_start(dst3[c], sb[:])  # scratch for hp1..3
                    (kbf_tiles if which == "k" else qbf_tiles)[c] = sb

                def pe_tpose(which, c):
                    # head-pair 0: PE transpose [128 t, 128 dd] -> [128 dd, 128 t]
                    sb = (kbf_tiles if which == "k" else qbf_tiles)[c]
                    dst = (kt if which == "k" else qt)[0]
                    tp = pst.tile([128, 128], BF16, tag="tptmp", name="tptmp")
                    nc.tensor.transpose(tp[:], sb[:, 0:128], ident[:])
                    nc.vector.tensor_copy(dst[:, c * 128 : (c + 1) * 128], tp[:])

                def v_chunk(c):
                    vs = stage.tile([128, DD], F32, tag="vstage", name=f"vs{c}")
                    nc.sync.dma_start(vs[:], v_3d[c])
                    nc.gpsimd.tensor_copy(
                        vc[c][:].rearrange("p (h e) -> p h e", e=65)[:, :, 0:64],
                        vs[:].rearrange("p (h e) -> p h e", e=64),
                    )

                def xbar_tposes(hp):
                    # head-pairs 1..3: transpose-load from bf16 scratch
                    csl = slice(hp * 128, (hp + 1) * 128)
                    for th in range(2):
                        tsl = slice(th * 1024, (th + 1) * 1024)
                        nc.sync.dma_start(kt[hp][:, tsl], kbf_d[tsl, csl], transpose=True)
                        nc.sync.dma_start(qt[hp][:, tsl], qbf_d[tsl, csl], transpose=True)

                # phase 1a: first t-half of Q/K; PE transposes for head-pair 0
                for c in range(8):
                    qk_chunk(k3s, k3d, c, "k")
                    qk_chunk(q3s, q3d, c, "q")
                    pe_tpose("k", c)
                    pe_tpose("q", c)
                for c in range(4):
                    v_chunk(c)
                # phase 1b: rest
                for c in range(8, 16):
                    qk_chunk(k3s, k3d, c, "k")
                    qk_chunk(q3s, q3d, c, "q")
                    pe_tpose("k", c)
                    pe_tpose("q", c)
                    v_chunk(c - 4)
                for c in range(12, 16):
                    v_chunk(c)
                xbar_tposes(1)

                # ---- main loop: one head PAIR at a time ----
                for hp in range(HP):
                    hA, hB = 2 * hp, 2 * hp + 1
                    if hp == 0:
                        xbar_tposes(2)
                    if hp == 1:
                        xbar_tposes(3)
                    for qq in range(NQQ):
                        qsl = slice(qq * QQ, (qq + 1) * QQ)
                        outpA = ps.tile([65, QQ], F32, tag="outpA", name="outpA")
                        outpB = ps.tile([65, QQ], F32, tag="outpB", name="outpB")
                        pbq = {}
                        for kci in range(KC + depth):
                            if kci < KC and "mm1" in stages:
                                kc = kci
                                # alternate PSUM bank groups so ACT exp
                                # reads never share a group with the next
                                # chunk's mm1 writes
                                pool = ps_sce if kc % 2 == 0 else ps_sco
                                sc = pool.tile([128, 1024], F32, tag="sc",
                                               name="sc")
                                ksl = slice(kc * 128, (kc + 1) * 128)
                                nc.tensor.matmul(
                                    sc[:, 0:512],
                                    kt[hp][0:64, ksl],
                                    qt[hp][0:64, qsl],
                                    start=True,
                                    stop=True,
                                    tile_position=(0, 0),
                                )
                                nc.tensor.matmul(
                                    sc[:, 512:1024],
                                    kt[hp][64:128, ksl],
                                    qt[hp][64:128, qsl],
                                    start=True,
                                    stop=True,
                                    tile_position=(64, 0),
                                )
                                if "exp" in stages:
                                    pb = probs_pool.tile(
                                        [128, 1024], BF16, tag="pb", name="pb"
                                    )
                                    nc.scalar.activation(
                                        pb[:], sc[:], EXP, scale=SCALE
                                    )
                                    pbq[kc] = pb
                            kc2 = kci - depth
                            if "mm2" in stages and 0 <= kc2 < KC:
                                pb2 = pbq.pop(kc2)
                                nc.tensor.matmul(
                                    outpA[:],
                                    vc[kc2][:, hA * 65 : (hA + 1) * 65],
                                    pb2[:, 0:512],
                                    start=(kc2 == 0),
                                    stop=(kc2 == KC - 1),
                                )
                                nc.tensor.matmul(
                                    outpB[:],
                                    vc[kc2][:, hB * 65 : (hB + 1) * 65],
                                    pb2[:, 512:1024],
                                    start=(kc2 == 0),
                                    stop=(kc2 == KC - 1),
                                )
                        if "mm2" not in stages or "norm" not in stages:
                            continue
                        for h, outp in ((hA, outpA), (hB, outpB)):
                            # evacuate outT from PSUM, then normalize from SBUF
                            outsb = norm_pool.tile(
                                [65, QQ], F32, tag="outsb", name="outsb"
                            )
                            nc.vector.tensor_copy(outsb[:], outp[:])
                            r = norm_pool.tile([1, QQ], F32, tag="r", name="r")
                            nc.vector.reciprocal(r[:], outsb[64:65, :])
                            bc = norm_pool.tile([64, QQ], F32, tag="bc", name="bc")
                            nc.gpsimd.partition_broadcast(bc[:], r[:])
                            ob = norm_pool.tile([64, QQ], F32, tag="ob", name="ob")
                            nc.vector.tensor_mul(ob[:], outsb[0:64, :], bc[:])
                            hi = norm_pool.tile([64, QQ], BF16, tag="hi", name="hi")
                            nc.gpsimd.tensor_copy(hi[:], ob[:])
                            lo = norm_pool.tile([64, QQ], BF16, tag="lo", name="lo")
                            nc.gpsimd.tensor_sub(lo[:], ob[:], hi[:])
                            hi_t = norm_pool.tile(
                                [128, 256], BF16, tag="hi_t", name="hi_t"
                            )
                            lo_t = norm_pool.tile(
                                [128, 256], BF16, tag="lo_t", name="lo_t"
                            )
                            nc.sync.dma_start(
                                hi_t[:].rearrange("p (m l) -> p m l", l=64),
                                hi[:],
                                transpose=True,
                            )
                            nc.sync.dma_start(
                                lo_t[:].rearrange("p (m l) -> p m l", l=64),
                                lo[:],
                                transpose=True,
                            )
                            ob2 = norm_pool.tile(
                                [128, 256], F32, tag="ob2", name="ob2"
                            )
                            nc.vector.tensor_add(ob2[:], hi_t[:], lo_t[:])
                            # out[qq*512 + m*128 + p, h*64 + d] <- ob2[p, m*64+d]
                            dest = o_d[
                                qsl, h * 64 : (h + 1) * 64
                            ].rearrange("(m p) d -> p m d", p=128)
                            nc.gpsimd.dma_start(
                                dest, ob2[:].rearrange("p (m l) -> p m l", l=64)
                            )
        if bench_iters:
            nc.sync.dma_start(out_flag[:], o_d[0:1, 0:1])
    return nc


_CACHED = None


def _get_program():
    global _CACHED
    if _CACHED is None:
        nc = bacc.Bacc("TRN2", target_bir_lowering=False, debug=False)
        _CACHED = build_program(nc)
        _CACHED.compile()
    return _CACHED


_IDENT = np.eye(128, dtype=ml_dtypes.bfloat16)


def _make_in_maps(querys, keys, values):
    querys = np.ascontiguousarray(np.asarray(querys, dtype=np.float32))
    keys = np.ascontiguousarray(np.asarray(keys, dtype=np.float32))
    values = np.ascontiguousarray(np.asarray(values, dtype=np.float32))
    in_maps = []
    for c in range(N_CORES):
        b, hb = c // 2, c % 2
        sl = slice(hb * DD, (hb + 1) * DD)
        in_maps.append(
            {
                "querys": querys[b, :, sl],
                "keys": keys[b, :, sl],
                "values": values[b, :, sl],
                "ident": _IDENT,
            }
        )
    return in_maps


def kernel(querys, keys, values):
    nc = _get_program()
    in_maps = _make_in_maps(querys, keys, values)
    res = run_bass_kernel_spmd(nc, in_maps, list(range(N_CORES)))
    out = np.empty((B, T, U), dtype=np.float32)
    for c in range(N_CORES):
        b, hb = c // 2, c % 2
        out[b, :, hb * DD : (hb + 1) * DD] = res.results[c]["out"]
    return out
